# revision 3
# baseline (speedup 1.0000x reference)
"""Causal self-attention (B=2, T=4096, C=768, H=12, D=64) on 8 trn2 cores.

Sharding: (B, H) -> 24 (batch, head) pairs, 3 heads per core.
Core c handles batch b = c // 4 and heads 3*(c%4) .. 3*(c%4)+2.
Each core computes the qkv projection for its heads, flash-style causal
attention (no running max; unnormalized P = exp(s/8), row sums via an
appended ones-column on V), and its partial output projection
(contraction over its 192 attn-output channels). The host sums the 4
partials per batch and adds the bias.

Per-core layouts:
  xT   [768, 4096]   x[b].T so the qk projection streams tokens in the free dim
  wqk  [768, 3, 128] col groups: [Wq_h0|Wq_h1], [Wk_h0|Wk_h1], [Wq_h2|Wk_h2]
  wvp  [768, 256]    [Wv_h0 Wv_h1 Wv_h2 0]
  wp   [3, 64, 768]  Wproj row chunk per head
  consts [128, 2240] 4 causal masks [128,512] + ones
Heads 0/1 are row-paired on the PE (head 0 in partitions 0-63, head 1 in
64-127) for the D=64-contraction score matmuls; head 2 runs solo in
partitions 0-63. Scores are computed transposed (ST[k, q]) so the PV
matmul contracts k on the partition dim with V in natural [t, d] layout.
"""

import numpy as np

B, T, C, H, D = 2, 4096, 768, 12, 64
HPC = 3          # heads per core
NCORES = 8
QB = 512         # query block (psum bank width in fp32)
NQB = T // QB    # 8
KT = 128         # key tile
NKT = T // KT    # 32
VSTRIDE = 200    # per-k-tile column stride in vbig (3*65 used + 5 pad)
NEG = -1.0e9

_COMPILED = {}
LAST = {}


def _emit(nc, tile, mybir, tc, ctx, aps, loop_reps=0, variant=""):
    F32 = mybir.dt.float32
    F32R = mybir.dt.float32r
    EXP = mybir.ActivationFunctionType.Exp
    xT, wqk, wvp, wp, consts, out = aps
    CC = C // 128  # 6 contraction chunks for the projections

    wpool = ctx.enter_context(tc.tile_pool(name="w", bufs=1))
    qkvpool = ctx.enter_context(tc.tile_pool(name="qkv", bufs=1))
    xpool = ctx.enter_context(tc.tile_pool(name="x", bufs=3))
    ptpool = ctx.enter_context(tc.tile_pool(name="pt", bufs=3))
    atpool = ctx.enter_context(tc.tile_pool(name="at", bufs=2))
    opool = ctx.enter_context(tc.tile_pool(name="osb", bufs=3))
    rpool = ctx.enter_context(tc.tile_pool(name="r", bufs=3))
    stp = ctx.enter_context(tc.tile_pool(name="stp", bufs=2, space="PSUM"))
    osp = ctx.enter_context(tc.tile_pool(name="osp", bufs=2, space="PSUM"))
    msp = ctx.enter_context(tc.tile_pool(name="msp", bufs=2, space="PSUM"))

    # ---- constants and weights ----
    masks_sb = wpool.tile([128, 2048], F32)
    nc.sync.dma_start(masks_sb[:], consts[:, 0:2048])
    ones64 = wpool.tile([1, 64], F32R)
    nc.sync.dma_start(ones64[:], consts[0:1, 2048:2112].bitcast(F32R))
    wqk_sb = wpool.tile([128, CC * 3 * 128], F32R)
    nc.sync.dma_start(
        wqk_sb[:].rearrange("p (a g m) -> p a g m", a=CC, g=3),
        wqk.bitcast(F32R).rearrange("(a p) g m -> p a g m", p=128),
    )
    wvp_sb = wpool.tile([128, CC * 256], F32R)
    nc.sync.dma_start(
        wvp_sb[:].rearrange("p (a n) -> p a n", a=CC),
        wvp.bitcast(F32R).rearrange("(a p) n -> p a n", p=128),
    )
    wp_sb = wpool.tile([64, 3 * C], F32R)
    nc.sync.dma_start(
        wp_sb[:].rearrange("p (g n) -> p g n", g=3),
        wp.bitcast(F32R).rearrange("g p n -> p g n"),
    )

    # ---- qkv storage ----
    # qkT01: [0:T] = qT (h0 rows 0-63, h1 rows 64-127), [T:2T] = kT
    qkT01 = qkvpool.tile([128, 2 * T], F32R)
    # qk2: rows 0-63 only: [0:T] = qT_h2, [T:2T] = kT_h2
    qk2 = qkvpool.tile([64, 2 * T], F32R)
    vbig = qkvpool.tile([128, NKT * VSTRIDE], F32R)
    vbig3 = vbig[:].rearrange("p (t c) -> p t c", c=VSTRIDE)
    # ones columns of vbig (col 65h+64 per k-tile), one DMA per head
    for h in range(3):
        nc.sync.dma_start(
            vbig3[:, :, 65 * h + 64 : 65 * h + 65],
            consts[:, 2112 + 32 * h : 2112 + 32 * (h + 1)]
            .bitcast(F32R)
            .rearrange("p (t u) -> p t u", u=1),
        )

    # ---- phase 1: qkv projections ----
    if loop_reps:
        loop_cm = tc.For_i(0, loop_reps, 1)
        loop_cm.__enter__()
    for tb in range(NQB):
        t0 = tb * QB
        xh = []
        for half in range(2):
            xt = xpool.tile([128, 3 * QB], F32R, tag="xt")
            nc.sync.dma_start(
                xt[:].rearrange("p (a t) -> p a t", a=3),
                xT[384 * half : 384 * (half + 1), t0 : t0 + QB]
                .bitcast(F32R)
                .rearrange("(a p) t -> p a t", p=128),
            )
            xh.append(xt)

        def xchunk(cc):
            return xh[cc // 3][:, (cc % 3) * QB : (cc % 3 + 1) * QB]

        # pair q then pair k: full 128-col stationary operand
        for g in range(2):
            ps = stp.tile([128, QB], F32, tag="st")
            for cc in range(CC):
                nc.tensor.matmul(
                    ps[:],
                    wqk_sb[:, (cc * 3 + g) * 128 : (cc * 3 + g + 1) * 128],
                    xchunk(cc),
                    start=(cc == 0),
                    stop=(cc == CC - 1),
                )
            nc.scalar.copy(qkT01[:, g * T + t0 : g * T + t0 + QB], ps[:])
        # head 2 q and k separately (M=64)
        for g2 in range(2):
            ps = osp.tile([64, QB], F32, tag="o")
            for cc in range(CC):
                base = (cc * 3 + 2) * 128 + 64 * g2
                nc.tensor.matmul(
                    ps[:],
                    wqk_sb[:, base : base + 64],
                    xchunk(cc),
                    start=(cc == 0),
                    stop=(cc == CC - 1),
                )
            nc.scalar.copy(qk2[:, g2 * T + t0 : g2 * T + t0 + QB], ps[:])
        # v: natural [t, d] layout, 4 k-tiles per tb
        for tt in range(4):
            kt = 4 * tb + tt
            ps = msp.tile([128, 256], F32, tag="m")
            for cc in range(CC):
                nc.tensor.matmul(
                    ps[:],
                    xchunk(cc)[:, tt * 128 : (tt + 1) * 128],
                    wvp_sb[:, cc * 256 : (cc + 1) * 256],
                    start=(cc == 0),
                    stop=(cc == CC - 1),
                )
            dst = vbig3[:, kt, 0:195].rearrange("p (h c) -> p h c", c=65)[:, :, 0:64]
            nc.scalar.copy(dst, ps[:, 0:192].rearrange("p (h d) -> p h d", h=3))

    # ---- phase 2: attention + output projection ----
    if "p1only" in variant:
        dummy = opool.tile([128, C], F32, tag="osb")
        nc.vector.tensor_copy(dummy[:], qkT01[:, 0:C].bitcast(F32))
        for tt in range(NQB * 4):
            nc.sync.dma_start(out[tt * 128 : (tt + 1) * 128, :], dummy[:])
        if loop_reps:
            loop_cm.__exit__(None, None, None)
        return
    def attend(qb, heads, att):
        """heads: list of (row_group, qT_ap, kT_ap, head_col)."""
        nkt = 4 * qb + 4
        t0 = qb * QB
        o_ps = [osp.tile([65, QB], F32, tag="o", name=f"ops{qb}_{i}") for i in range(len(heads))]
        for g in range(nkt // 2):
            sts = [stp.tile([128, 2 * QB], F32, tag="st", name=f"st{qb}_{g}_{i}") for i in range(len(heads))]
            for i in range(2):
                kt = 2 * g + i
                for hh, (rg, qT_ap, kT_ap, hcol) in enumerate(heads):
                    nc.tensor.matmul(
                        sts[hh][:, i * QB : (i + 1) * QB],
                        kT_ap[:, kt * KT : (kt + 1) * KT],
                        qT_ap[:, t0 : t0 + QB],
                        start=True,
                        stop=True,
                        tile_position=(64 * rg, 0),
                    )
            for hh, (rg, qT_ap, kT_ap, hcol) in enumerate(heads):
                st = sts[hh]
                pt = ptpool.tile([128, 2 * QB], F32R, tag="pt")
                if "dveexp" in variant:
                    nc.vector.tensor_copy(pt[:], st[:])
                else:
                    nc.scalar.activation(pt[:], st[:], EXP, scale=float(D) ** -0.5)
                if "nomask" not in variant:
                    dg = g - 2 * qb
                    if dg >= 0:
                        nc.vector.tensor_mul(
                            pt[:], pt[:], masks_sb[:, dg * 1024 : (dg + 1) * 1024]
                        )
                if "noav" in variant:
                    if g == 0:
                        nc.tensor.matmul(
                            o_ps[hh][:],
                            vbig3[:, 0, 65 * hcol : 65 * hcol + 65],
                            pt[:, 0:QB],
                            start=True,
                            stop=True,
                        )
                else:
                    for i in range(2):
                        kt = 2 * g + i
                        nc.tensor.matmul(
                            o_ps[hh][:],
                            vbig3[:, kt, 65 * hcol : 65 * hcol + 65],
                            pt[:, i * QB : (i + 1) * QB],
                            start=(kt == 0),
                            stop=(kt == nkt - 1),
                        )
        for hh, (rg, qT_ap, kT_ap, hcol) in enumerate(heads):
            asl = att[:, hcol * QB : (hcol + 1) * QB]
            nc.vector.tensor_copy(asl, o_ps[hh][0:64, :])
            rs = rpool.tile([1, QB], F32R, tag="r")
            with nc.allow_low_precision(reason="f32r recip feeds f32r matmul"):
                nc.vector.reciprocal(rs[:], o_ps[hh][64:65, :])
            bc = msp.tile([64, QB], F32, tag="m")
            nc.tensor.matmul(bc[:], ones64[:], rs[:], start=True, stop=True)
            nc.vector.tensor_mul(asl, asl, bc[:])

    for qb in range(NQB):
        t0 = qb * QB
        att = atpool.tile([64, 3 * QB], F32R, tag="att")
        attend(
            qb,
            [
                (0, qkT01[0:64, 0:T], qkT01[0:64, T : 2 * T], 0),
                (1, qkT01[64:128, 0:T], qkT01[64:128, T : 2 * T], 1),
            ],
            att,
        )
        attend(qb, [(0, qk2[:, 0:T], qk2[:, T : 2 * T], 2)], att)
        # output projection for this query block
        for tt in range(4):
            osb = opool.tile([128, C], F32, tag="osb")
            for j in range(2):
                pps = msp.tile([128, 384], F32, tag="m")
                for h in range(3):
                    nc.tensor.matmul(
                        pps[:],
                        att[:, h * QB + tt * 128 : h * QB + (tt + 1) * 128],
                        wp_sb[:, h * C + 384 * j : h * C + 384 * (j + 1)],
                        start=(h == 0),
                        stop=(h == 2),
                    )
                nc.vector.tensor_copy(osb[:, 384 * j : 384 * (j + 1)], pps[:])
            nc.sync.dma_start(out[t0 + tt * 128 : t0 + (tt + 1) * 128, :], osb[:])
    if loop_reps:
        loop_cm.__exit__(None, None, None)


def _build(loop_reps=0, variant=""):
    import concourse.bass as bass  # noqa: F401
    import concourse.tile as tile
    import concourse.mybir as mybir
    from concourse import bacc
    from contextlib import ExitStack

    F32 = mybir.dt.float32
    nc = bacc.Bacc()
    xT = nc.dram_tensor("xT", [C, T], F32, kind="ExternalInput").ap()
    wqk = nc.dram_tensor("wqk", [C, 3, 128], F32, kind="ExternalInput").ap()
    wvp = nc.dram_tensor("wvp", [C, 256], F32, kind="ExternalInput").ap()
    wp = nc.dram_tensor("wp", [3, 64, C], F32, kind="ExternalInput").ap()
    consts = nc.dram_tensor("consts", [128, 2240], F32, kind="ExternalInput").ap()
    out = nc.dram_tensor("out", [T, C], F32, kind="ExternalOutput").ap()

    with tile.TileContext(nc) as tc, ExitStack() as ctx:
        _emit(nc, tile, mybir, tc, ctx, (xT, wqk, wvp, wp, consts, out), loop_reps, variant)
    nc.compile()
    return nc


def _consts_np():
    consts = np.zeros((128, 2240), np.float32)
    p = np.arange(128)[:, None]
    f = np.arange(512)[None, :]
    for m in range(4):
        rel = f - 128 * m
        mask = np.where(rel < 128, np.where(rel >= p, 1.0, 0.0), 1.0)
        mask = np.where(rel < 0, 0.0, mask)
        consts[:, m * 512 : (m + 1) * 512] = mask
    consts[:, 2048:2240] = 1.0
    return consts


def _shard_inputs(x, Wqkv, Wproj):
    consts = _consts_np()
    in_maps = []
    for c in range(NCORES):
        b = c // 4
        hs = [3 * (c % 4) + j for j in range(HPC)]
        wqk = np.zeros((C, 3, 128), np.float32)
        wqk[:, 0, 0:64] = Wqkv[:, (0 * H + hs[0]) * D : (0 * H + hs[0] + 1) * D]
        wqk[:, 0, 64:128] = Wqkv[:, (0 * H + hs[1]) * D : (0 * H + hs[1] + 1) * D]
        wqk[:, 1, 0:64] = Wqkv[:, (1 * H + hs[0]) * D : (1 * H + hs[0] + 1) * D]
        wqk[:, 1, 64:128] = Wqkv[:, (1 * H + hs[1]) * D : (1 * H + hs[1] + 1) * D]
        wqk[:, 2, 0:64] = Wqkv[:, (0 * H + hs[2]) * D : (0 * H + hs[2] + 1) * D]
        wqk[:, 2, 64:128] = Wqkv[:, (1 * H + hs[2]) * D : (1 * H + hs[2] + 1) * D]
        wvp = np.zeros((C, 256), np.float32)
        for j, h in enumerate(hs):
            wvp[:, j * 64 : (j + 1) * 64] = Wqkv[
                :, (2 * H + h) * D : (2 * H + h + 1) * D
            ]
        wp = np.stack([Wproj[h * D : (h + 1) * D, :] for h in hs]).astype(np.float32)
        in_maps.append(
            {
                "xT": np.ascontiguousarray(x[b].T),
                "wqk": wqk,
                "wvp": wvp,
                "wp": wp,
                "consts": consts,
            }
        )
    return in_maps


def kernel(x, Wqkv, Wproj, bproj):
    from concourse.bass_utils import run_bass_kernel_spmd

    x = np.asarray(x, np.float32)
    Wqkv = np.asarray(Wqkv, np.float32)
    Wproj = np.asarray(Wproj, np.float32)
    bproj = np.asarray(bproj, np.float32)

    if "nc" not in _COMPILED:
        _COMPILED["nc"] = _build()
    nc = _COMPILED["nc"]

    in_maps = _shard_inputs(x, Wqkv, Wproj)
    r = run_bass_kernel_spmd(nc, in_maps, list(range(NCORES)))
    LAST["res"] = r
    res = r.results
    out = np.zeros((B, T, C), np.float32)
    for c in range(NCORES):
        out[c // 4] += res[c]["out"]
    out += bproj[None, None, :]
    return out

